# revision 1
# baseline (speedup 1.0000x reference)
"""Causal multi-head attention (B=4, L=2048, D=1024, H=16) on 8 TRN2 NeuronCores.

Sharding: core = (batch b, head-group hg) with b in 0..3, hg in 0..1.
Each core computes, for its batch and its 8 heads:
  qT/kT = (Wq_hg x_b^T), v = x_b Wv_hg^T          (f32r matmuls)
  scoresT[kp,qp] = kT^T qT per head (causal col ranges, f32r,
      two heads packed onto PE partition halves and run concurrently)
  attnT = exp(scoresT/8) (ScalarE, bf16 out, fused PSUM evac), diag masked
  outT[d,qp] (+denominator row via ones-column in v_aug)  (bf16 matmuls)
  outT normalized by 1/denom, partial = outT^T Wo_hg^T    (f32r)
Host sums the two head-group partials per batch.

The attention inner loop is software-pipelined per head pair: the attn@v
chains for qp-block b depend only on kp-chunks j <= 4b+3, so they are
emitted right after that j-group's scores/exp — PE (attn@v) runs
concurrently with ScalarE (exp of the next j-group), which measured 1.56x
faster than phase-sequential emission.

kernel(**inputs) takes the full unsharded inputs and returns the full output.
"""

import numpy as np

import concourse.bass as bass
import concourse.mybir as mybir
import concourse.tile as tile
from concourse import bacc
from concourse.bass_utils import run_bass_kernel_spmd

F32 = mybir.dt.float32
F32R = mybir.dt.float32r
BF16 = mybir.dt.bfloat16

L = 2048          # sequence length
D = 1024          # model dim
HG = 8            # heads per core
DH = 64           # head dim
DHG = HG * DH     # 512, head-group width
DC = D // 128     # 8 contraction chunks for projections
LT = L // 128     # 16 key-position chunks
QB = L // 512     # 4 query blocks of 512
N_CORES = 8


def _ceil_div(a, b):
    return (a + b - 1) // b


# column offset of kp-chunk j's storage inside the packed causal attnT buffer
def _off(j):
    return 2048 * j - 128 * (j * (j - 1) // 2)


ATT_W = _off(LT)  # 17408 packed causal columns per head


SKIP = set()


def build_kernel(reps: int = 0, phases: str = "pao"):
    """Build the SPMD Bass program. reps>0 wraps the body in a hardware loop
    (body executed reps+1 times total) for timing."""
    nc = bacc.Bacc()

    xT = nc.dram_tensor("xT", [D, L], F32R, kind="ExternalInput")
    wqT = nc.dram_tensor("wqT", [D, DHG], F32R, kind="ExternalInput")
    wkT = nc.dram_tensor("wkT", [D, DHG], F32R, kind="ExternalInput")
    wvT = nc.dram_tensor("wvT", [D, DHG], F32R, kind="ExternalInput")
    woT = nc.dram_tensor("woT", [DHG, D], F32R, kind="ExternalInput")
    out = nc.dram_tensor("out", [L, D], F32, kind="ExternalOutput")
    dscr = nc.dram_tensor("dscr", [HG, QB, 512], F32)  # denom recip scratch

    xT_r = xT[:, :].rearrange("(c p) l -> p c l", p=128)
    wqT_r = wqT[:, :].rearrange("(c p) m -> p c m", p=128)
    wkT_r = wkT[:, :].rearrange("(c p) m -> p c m", p=128)
    wvT_r = wvT[:, :].rearrange("(c p) m -> p c m", p=128)
    woT_r = woT[:, :].rearrange("(c p) n -> p c n", p=128)
    out_r = out[:, :].rearrange("(t p) n -> p t n", p=128)

    with tile.TileContext(nc) as tc:
        ctx_body(nc, tc, xT_r, wqT_r, wkT_r, wvT_r, woT_r, out_r, dscr, reps, phases)
    nc.compile()
    return nc


def ctx_body(nc, tc, xT_r, wqT_r, wkT_r, wvT_r, woT_r, out_r, dscr, reps, phases="pao"):
    from contextlib import ExitStack

    with ExitStack() as es:
        persist = es.enter_context(tc.tile_pool(name="persist", bufs=1))
        # persistent tiles (live across the whole body)
        qT_sb = persist.tile([128, 4, L], F32R)   # head h: partitions (h%2)*64.., slot h//2
        kT_sb = persist.tile([128, 4, L], F32R)
        v_sb = persist.tile([128, LT, HG, DH + 1], BF16)  # v + ones column
        mask_sb = persist.tile([128, 128], BF16)  # upper-tri (incl diag) ones

        # constant setup (outside the timing loop)
        # mask[kp, qp] = 1 where kp <= qp else 0: keep where (qp - kp) >= 0
        nc.gpsimd.memset(mask_sb, 1.0)
        nc.gpsimd.affine_select(
            out=mask_sb,
            in_=mask_sb,
            compare_op=mybir.AluOpType.is_ge,
            fill=0.0,
            base=0,
            pattern=[[1, 128]],
            channel_multiplier=-1,
        )
        # ones column of v_aug
        nc.vector.memset(v_sb[:, :, :, DH : DH + 1], 1.0)

        def body():
            with ExitStack() as bs:
                proj = bs.enter_context(tc.tile_pool(name="proj", bufs=1))
                pj_ps = bs.enter_context(
                    tc.tile_pool(name="pj_ps", bufs=8, space="PSUM")
                )
                xT_sb = proj.tile([128, DC, L], F32R)
                wq_sb = proj.tile([128, DC, DHG], F32R)
                wk_sb = proj.tile([128, DC, DHG], F32R)
                wv_sb = proj.tile([128, DC, DHG], F32R)
                for c in range(DC):
                    nc.sync.dma_start(out=wq_sb[:, c, :], in_=wqT_r[:, c, :])
                    nc.sync.dma_start(out=wk_sb[:, c, :], in_=wkT_r[:, c, :])
                    nc.sync.dma_start(out=wv_sb[:, c, :], in_=wvT_r[:, c, :])
                    nc.sync.dma_start(out=xT_sb[:, c, :], in_=xT_r[:, c, :])

                # qT / kT projections: stationary = W chunk, moving = xT
                for w_sb, dst in () if "pjmm" in SKIP else ((wq_sb, qT_sb), (wk_sb, kT_sb)):
                    for t in range(4):  # output dq tile (2 heads)
                        for qb in range(QB):
                            ps = pj_ps.tile([128, 512], F32, tag="pj")
                            for c in range(DC):
                                nc.tensor.matmul(
                                    ps,
                                    w_sb[:, c, t * 128 : (t + 1) * 128],
                                    xT_sb[:, c, qb * 512 : (qb + 1) * 512],
                                    start=(c == 0),
                                    stop=(c == DC - 1),
                                )
                            nc.vector.tensor_copy(
                                dst[:, t, qb * 512 : (qb + 1) * 512], ps
                            )

                # v projection (natural layout): stationary = xT chunk
                for it in () if "pjmm" in SKIP else range(LT):
                    ps = pj_ps.tile([128, 512], F32, tag="pj")
                    for c in range(DC):
                        nc.tensor.matmul(
                            ps,
                            xT_sb[:, c, it * 128 : (it + 1) * 128],
                            wv_sb[:, c, :],
                            start=(c == 0),
                            stop=(c == DC - 1),
                        )
                    nc.vector.tensor_copy(
                        v_sb[:, it, :, 0:DH],
                        ps.rearrange("p (h d) -> p h d", h=HG),
                    )

            if "pjmm" not in SKIP and ("a" not in phases or "sc" in SKIP):
                # keep proj live when its consumers are ablated
                nc.sync.dma_start(out=out_r[:, 0, 0:512], in_=qT_sb[:, 0, 0:512].bitcast(F32))
                nc.sync.dma_start(out=out_r[:, 1, 0:512], in_=kT_sb[:, 0, 0:512].bitcast(F32))
                nc.gpsimd.dma_start(
                    out=out_r[:, 2, 0:520], in_=v_sb[:, 0, :, :]
                )
            if "a" not in phases:
                return
            with ExitStack() as ms:
                mid = ms.enter_context(tc.tile_pool(name="mid", bufs=1))
                outT_sb = mid.tile([128, 4, L], F32R)
                attn_phase(ms, outT_sb)
                if "o" in phases:
                    outproj_phase(ms, outT_sb)

        def attn_phase(ms, outT_sb):
            # attention, one head pair at a time
            with ExitStack() as bs:
                att = bs.enter_context(tc.tile_pool(name="att", bufs=2))
                # one shared 8-bank psum pool: the scores stretch and the
                # attn@v stretch alternate, so each gets the full depth
                sc_ps = bs.enter_context(
                    tc.tile_pool(name="sc_ps", bufs=8, space="PSUM")
                )
                oT_ps = sc_ps
                nrm = bs.enter_context(tc.tile_pool(name="nrm", bufs=2))

                for hp in range(4):  # head pair (2*hp, 2*hp+1)
                    at0 = att.tile([128, ATT_W], BF16, tag="attnT")
                    at1 = att.tile([128, ATT_W], BF16, tag="attnT")
                    atl = (at0, at1)
                    # Software-pipelined within the pair: once j-group
                    # 4b..4b+3 is exp'd, the attn@v chains for qp-block b are
                    # ready, so they are emitted immediately -- PE (attn@v)
                    # overlaps ScalarE (exp of the following j-group).
                    # scoresT: even head on partitions 0:64, odd on 64:128
                    # (PE row-packing, concurrent tiles).
                    for b in range(QB):
                        for j in range(4 * b, 4 * b + 4):
                            ncols = L - 128 * j
                            for c0 in range(0, ncols, 512):
                                w = min(512, ncols - c0)
                                for hh in () if "sc" in SKIP else range(2):
                                    p0 = hh * 64
                                    ps = sc_ps.tile([128, 512], F32, tag="ps8")
                                    nc.tensor.matmul(
                                        ps[:, :w],
                                        kT_sb[p0 : p0 + 64, hp, j * 128 : (j + 1) * 128],
                                        qT_sb[p0 : p0 + 64, hp, 128 * j + c0 : 128 * j + c0 + w],
                                        start=True,
                                        stop=True,
                                    )
                                    if "exp" not in SKIP:
                                        nc.scalar.activation(
                                            atl[hh][:, _off(j) + c0 : _off(j) + c0 + w],
                                            ps[:, :w],
                                            mybir.ActivationFunctionType.Exp,
                                            scale=0.125,
                                        )
                            # mask the diagonal block (first 128 stored cols of j)
                            for hh in range(2):
                                nc.vector.tensor_mul(
                                    atl[hh][:, _off(j) : _off(j) + 128],
                                    atl[hh][:, _off(j) : _off(j) + 128],
                                    mask_sb,
                                )
                        # attn @ v_aug -> outT [65, qp] for qp-block b, both heads
                        for hh in () if "av" in SKIP else range(2):
                            h = 2 * hp + hh
                            ps = oT_ps.tile([128, 512], F32, tag="ps8")
                            jmax = 4 * b + 3
                            for j in range(jmax + 1):
                                qp0 = 512 * b
                                lo = max(qp0, 128 * j)
                                w = 512 * b + 512 - lo
                                nc.tensor.matmul(
                                    ps[0 : DH + 1, lo - qp0 : 512],
                                    v_sb[:, j, h, :],
                                    atl[hh][
                                        :, _off(j) + lo - 128 * j : _off(j) + lo - 128 * j + w
                                    ],
                                    start=(j == 0),
                                    stop=(j == jmax),
                                )
                            # evacuate PSUM immediately (frees the bank), then
                            # normalize decoupled: recip -> DRAM ->
                            # partition-broadcast load -> multiply. DVE lanes
                            # cannot shift partitions, so odd heads (target
                            # partitions 64:128) go via staging + SBUF DMA.
                            ust = nrm.tile([128, 512], F32, tag="ust", bufs=4)
                            nc.vector.tensor_copy(ust[0 : DH + 1, :], ps[0 : DH + 1, :])
                            recip = nrm.tile([128, 512], F32, tag="recip")
                            rep = nrm.tile([128, 512], F32, tag="rep")
                            nc.vector.reciprocal(
                                recip[DH : DH + 1, :], ust[DH : DH + 1, :]
                            )
                            nc.sync.dma_start(
                                out=dscr[h, b, :], in_=recip[DH : DH + 1, :]
                            )
                            dsrc = dscr[h, b, :]
                            nc.sync.dma_start(
                                out=rep[0:DH, :],
                                in_=bass.AP(
                                    tensor=dsrc.tensor,
                                    offset=dsrc.offset,
                                    ap=[[0, DH]] + [list(p) for p in dsrc.ap],
                                ),
                            )
                            if hh == 0:
                                nc.vector.tensor_mul(
                                    outT_sb[0:DH, hp, b * 512 : (b + 1) * 512],
                                    ust[0:DH, :],
                                    rep[0:DH, :],
                                )
                            else:
                                stg = nrm.tile([128, 512], F32R, tag="stg")
                                nc.vector.tensor_mul(
                                    stg[0:DH, :], ust[0:DH, :], rep[0:DH, :]
                                )
                                nc.sync.dma_start(
                                    out=outT_sb[
                                        DH:128, hp, b * 512 : (b + 1) * 512
                                    ],
                                    in_=stg[0:DH, :],
                                )

        def outproj_phase(ms, outT_sb):
            # out-projection: partial[qp, :] = outT^T @ woT
            with ExitStack() as bs:
                op_ps = bs.enter_context(
                    tc.tile_pool(name="op_ps", bufs=4, space="PSUM")
                )
                wop = bs.enter_context(tc.tile_pool(name="wop", bufs=1))
                ost = bs.enter_context(tc.tile_pool(name="ost", bufs=3))
                wo_sb = wop.tile([128, 4, D], F32R)
                nc.sync.dma_start(out=wo_sb, in_=woT_r)
                for qt in range(LT):
                    ps = op_ps.tile([128, D], F32, tag="op")
                    for c in range(4):
                        for nh in range(2):
                            nc.tensor.matmul(
                                ps[:, nh * 512 : (nh + 1) * 512],
                                outT_sb[:, c, qt * 128 : (qt + 1) * 128],
                                wo_sb[:, c, nh * 512 : (nh + 1) * 512],
                                start=(c == 0),
                                stop=(c == 3),
                            )
                    ot = ost.tile([128, D], F32, tag="ot")
                    nc.vector.tensor_copy(ot, ps)
                    nc.sync.dma_start(out=out_r[:, qt, :], in_=ot)

        if reps > 0:
            with tc.For_i(0, reps):
                body()
        body()


_CACHE = {}


def _get_runner(reps=0):
    if reps not in _CACHE:
        _CACHE[reps] = build_kernel(reps)
    return _CACHE[reps]


def make_in_maps(x, Wq, Wk, Wv, Wo):
    in_maps = []
    for core in range(N_CORES):
        b, hg = divmod(core, 2)
        sl = slice(hg * DHG, (hg + 1) * DHG)
        in_maps.append(
            {
                "xT": np.ascontiguousarray(x[b].T.astype(np.float32)),
                "wqT": np.ascontiguousarray(Wq[sl, :].T.astype(np.float32)),
                "wkT": np.ascontiguousarray(Wk[sl, :].T.astype(np.float32)),
                "wvT": np.ascontiguousarray(Wv[sl, :].T.astype(np.float32)),
                "woT": np.ascontiguousarray(Wo[:, sl].T.astype(np.float32)),
            }
        )
    return in_maps


def kernel(x, Wq, Wk, Wv, Wo):
    x = np.asarray(x)
    nc = _get_runner(0)
    in_maps = make_in_maps(x, Wq, Wk, Wv, Wo)
    res = run_bass_kernel_spmd(nc, in_maps, core_ids=list(range(N_CORES)))
    B = x.shape[0]
    out = np.empty((B, L, D), dtype=np.float32)
    for b in range(B):
        out[b] = res.results[2 * b]["out"] + res.results[2 * b + 1]["out"]
    return out

